# revision 11
# baseline (speedup 1.0000x reference)
"""Trainium2 Bass kernel for nn_Bert_BiLSTM (segment-mean pooling + BiLSTM).

Strategy (8 NeuronCores, data-parallel over batch, Bc=8 per core):
  Phase A (pooling): pooledT[d,w] = hidden[t,d]^T @ M_scaled[t,w] via fp32r
      matmuls, where M_scaled is the host-built one-hot(word_ids)/count
      matrix (index preprocessing only).
  Phase B (projection): pre[g,w] = w_ih^T @ pooledT (fp32r) + bias, stored
      bf16. Computed in w-halves ordered so the scan can start after the
      first two halves; the last two halves overlap the scan.
  Phase C (scan): 256 sequential LSTM steps per direction, both directions
      interleaved on each core. Gates in [G-part, B-free] layout: w_hh
      tiles (bf16) stationary, h (bf16) moving. pre_t is injected into
      PSUM via an identity matmul (opens the accumulation group), so the
      gates emerge complete in PSUM and ACT reads them directly.
      Elementwise in fp32 on DVE/ACT/GpSimd.
  Phase D: PE-transpose h history to [w, h] layout and DMA out.

Host side: shard batch, build M_scaled, permute gates to [i,f,o,g] order,
cast weights, assemble/concat outputs.
"""

import os
import sys

for _p in ("/opt/trn_rl_repo", "/root/.axon_site/_ro/trn_rl_repo"):
    if os.path.isdir(_p) and _p not in sys.path:
        sys.path.append(_p)

import numpy as np
import ml_dtypes

NCORES = 8
BC = 8          # batch per core
T = 512
D = 768
W = 256
H = 256
G = 1024        # 4*H
NT = T // 128   # 4 t-tiles
ND = D // 128   # 6 d-chunks
NG = G // 128   # 8 gate chunks (per direction)
KT = H // 128   # 2 h-chunks

_NC_CACHE = {}


def build_nc():
    """Build and compile the per-core Bass program (SPMD, same on all cores)."""
    import concourse.bacc as bacc
    import concourse.tile as tile
    from concourse import mybir
    from concourse.masks import make_identity

    f32 = mybir.dt.float32
    f32r = mybir.dt.float32r
    bf16 = mybir.dt.bfloat16
    AF = mybir.ActivationFunctionType
    ALU = mybir.AluOpType

    nc = bacc.Bacc("TRN2", target_bir_lowering=False, debug=False,
                   enable_asserts=False, num_devices=NCORES)

    hs = nc.dram_tensor("hs", [BC, NT, 128, D], f32r, kind="ExternalInput")
    msc = nc.dram_tensor("msc", [BC, NT, 128, W], f32r, kind="ExternalInput")
    wih = nc.dram_tensor("wih", [2, ND, 128, G], f32r, kind="ExternalInput")
    whh = nc.dram_tensor("whh", [2, KT, 128, G], bf16, kind="ExternalInput")
    bias = nc.dram_tensor("bias", [2 * NG, 128], f32, kind="ExternalInput")
    outf = nc.dram_tensor("outf", [BC, W, H], f32, kind="ExternalOutput")
    outb = nc.dram_tensor("outb", [BC, W, H], f32, kind="ExternalOutput")

    with tile.TileContext(nc) as tc:
        from contextlib import ExitStack
        ctx = ExitStack()
        with ctx:
            const = ctx.enter_context(tc.tile_pool(name="const", bufs=1))
            whh_sb = const.tile([128, 2, KT, G], bf16)
            nc.sync.dma_start(out=whh_sb, in_=whh.ap().rearrange("d k p g -> p d k g"))
            bias_sb = const.tile([128, 2 * NG], f32)
            nc.sync.dma_start(out=bias_sb, in_=bias.ap().rearrange("n p -> p n"))
            ident = const.tile([128, 128], bf16)
            make_identity(nc, ident)
            ident_pre = const.tile([128, 128], bf16)
            make_identity(nc, ident_pre)

            pooledT = const.tile([128, BC, ND, W], f32r)    # 48KB/part
            pre = const.tile([128, 2, W, NG, BC], bf16)     # 64KB/part
            hh = const.tile([128, 2, KT, BC, W + 1], bf16)  # h history
            cc = const.tile([128, 2, KT, BC], f32)

            # ---- Phase A: pooling ----
            with tc.tile_pool(name="hsst", bufs=2) as hsp, \
                 tc.tile_pool(name="mscst", bufs=2) as mscp, \
                 tc.tile_pool(name="psA", bufs=6, space="PSUM") as psA:
                for b in range(BC):
                    hst = []
                    msct = []
                    for tt in range(NT):
                        ht = hsp.tile([128, D], f32r, tag=f"hs{tt}")
                        nc.sync.dma_start(out=ht, in_=hs.ap()[b, tt])
                        hst.append(ht)
                        mt = mscp.tile([128, W], f32r, tag=f"ms{tt}")
                        nc.sync.dma_start(out=mt, in_=msc.ap()[b, tt])
                        msct.append(mt)
                    for dc in range(ND):
                        pps = psA.tile([128, W], f32)
                        for tt in range(NT):
                            nc.tensor.matmul(
                                out=pps,
                                lhsT=hst[tt][:, dc * 128:(dc + 1) * 128],
                                rhs=msct[tt],
                                start=(tt == 0), stop=(tt == NT - 1))
                        if (b * ND + dc) % 2 == 0:
                            nc.scalar.copy(pooledT[:, b, dc, :], pps)
                        else:
                            nc.vector.tensor_copy(pooledT[:, b, dc, :], pps)

            # scan pools first so the proj/psD pool stacks can close in
            # LIFO order around them
            bc_ctx = ctx.enter_context(ExitStack())
            psC = bc_ctx.enter_context(tc.tile_pool(name="psC", bufs=3, space="PSUM"))
            sp = bc_ctx.enter_context(tc.tile_pool(name="sp", bufs=3))
            gp = bc_ctx.enter_context(tc.tile_pool(name="gp", bufs=3))
            tp = bc_ctx.enter_context(tc.tile_pool(name="tp", bufs=3))
            thp = bc_ctx.enter_context(tc.tile_pool(name="thp", bufs=3))

            # ---- Phase B: projection in w-halves; scan interleaved ----
            pb_ctx = ExitStack()
            wihp = pb_ctx.enter_context(tc.tile_pool(name="wihp", bufs=1))
            psB = pb_ctx.enter_context(tc.tile_pool(name="psB", bufs=1, space="PSUM"))

            def proj_half(di, wh):
                wih_sb = wihp.tile([128, ND, G], f32r, tag="wih")
                nc.sync.dma_start(
                    out=wih_sb, in_=wih.ap()[di].rearrange("c p g -> p c g"))
                for gc in range(NG):
                    ppj = psB.tile([128, BC, 128], f32)   # 2 banks
                    for dc in range(ND):
                        for bq in range(2):
                            nc.tensor.matmul(
                                out=ppj[:, 4 * bq:4 * bq + 4, :],
                                lhsT=wih_sb[:, dc, gc * 128:(gc + 1) * 128],
                                rhs=pooledT[:, 4 * bq:4 * bq + 4, dc,
                                            wh * 128:(wh + 1) * 128],
                                start=(dc == 0), stop=(dc == ND - 1))
                    bcol = bias_sb[:, di * NG + gc: di * NG + gc + 1]
                    # pre is (w, gc, b)-ordered; psum is (b, w): permute APs
                    dst = pre[:, di, wh * 128:(wh + 1) * 128, gc, :]
                    src_ap = ppj.rearrange("p b w -> p w b")
                    if gc % 2 == 0:
                        nc.scalar.activation(dst, src_ap, AF.Identity,
                                             bias=bcol, scale=1.0)
                    else:
                        nc.vector.tensor_scalar(dst, src_ap, bcol, None, ALU.add)

            # ---- Phase C: the LSTM scan ----
            nc.vector.memset(hh[:, 0, :, :, 0], 0.0)     # fwd h_{-1} = 0
            nc.vector.memset(hh[:, 1, :, :, W], 0.0)     # bwd h_{W} = 0
            nc.vector.memset(cc, 0.0)

            def scan_mm(t, di):
                tf = t if di == 0 else W - 1 - t
                rslot = tf if di == 0 else tf + 1
                wslot = tf + 1 if di == 0 else tf
                # g gates (tanh) in their own bank so tanh starts early
                ps_g = psC.tile([128, KT, BC], f32, tag="psg")
                nc.tensor.matmul(out=ps_g, lhsT=ident_pre,
                                 rhs=pre[:, di, tf, 6:8, :],
                                 start=True, stop=False)
                # kt-outer: all k0 matmuls first so they're gated only on
                # the h0 half-write of the previous step
                for kt in range(KT):
                    for j, gc in enumerate((6, 7)):
                        nc.tensor.matmul(
                            out=ps_g[:, j, :],
                            lhsT=whh_sb[:, di, kt, gc * 128:(gc + 1) * 128],
                            rhs=hh[:, di, kt, :, rslot],
                            start=False, stop=(gc == 7 and kt == KT - 1))
                # i,f,o gates (sigmoid) in a second bank
                ps_s = psC.tile([128, 6, BC], f32, tag="psifo")
                nc.tensor.matmul(out=ps_s, lhsT=ident_pre,
                                 rhs=pre[:, di, tf, 0:6, :],
                                 start=True, stop=False)
                for kt in range(KT):
                    for gc in range(6):
                        nc.tensor.matmul(
                            out=ps_s[:, gc, :],
                            lhsT=whh_sb[:, di, kt, gc * 128:(gc + 1) * 128],
                            rhs=hh[:, di, kt, :, rslot],
                            start=False, stop=(gc == 5 and kt == KT - 1))
                return (di, ps_g, ps_s, wslot)

            def scan_ew(st):
                di, ps_g, ps_s, wslot = st
                g = gp.tile([128, KT, BC], f32)
                nc.scalar.activation(g, ps_g, AF.Tanh)
                s = sp.tile([128, 6, BC], f32)
                nc.scalar.activation(s, ps_s, AF.Sigmoid)
                tmp = tp.tile([128, KT, BC], f32)
                nc.gpsimd.tensor_mul(tmp, s[:, 0:2, :], g)
                nc.vector.tensor_mul(cc[:, di], s[:, 2:4, :], cc[:, di])
                nc.vector.tensor_add(cc[:, di], cc[:, di], tmp)
                th = thp.tile([128, KT, BC], f32)
                nc.scalar.activation(th, cc[:, di], AF.Tanh)
                # split h write by kt half: next step's k0 matmuls only wait
                # for the first half
                nc.vector.tensor_mul(hh[:, di, 0, :, wslot], s[:, 4, :], th[:, 0, :])
                nc.vector.tensor_mul(hh[:, di, 1, :, wslot], s[:, 5, :], th[:, 1, :])

            # Anti-phase emission: bwd's elementwise is emitted alongside
            # fwd's matmul burst and vice versa, so each chain's PE burst
            # hides under the other chain's ACT/DVE tail.
            pend_b = None
            proj_half(0, 0)   # fwd, w in [0,128)
            proj_half(1, 1)   # bwd, w in [128,256)
            for t in range(W // 2):
                st_f = scan_mm(t, 0)
                if pend_b is not None:
                    scan_ew(pend_b)
                scan_ew(st_f)
                pend_b = scan_mm(t, 1)
            scan_ew(pend_b)
            pend_b = None
            proj_half(0, 1)
            proj_half(1, 0)
            pb_ctx.close()

            # ---- Phase D (part 1): output chunks finished by scan half 1,
            # emitted here so they fill engine gaps during scan half 2 ----
            psD = bc_ctx.enter_context(tc.tile_pool(name="psD", bufs=2, space="PSUM"))
            stg = bc_ctx.enter_context(tc.tile_pool(name="stg", bufs=4))

            def emit_out(di, b, wc):
                odram = outf if di == 0 else outb
                base = 1 if di == 0 else 0
                pst = psD.tile([128, KT, 128], bf16)
                for kt in range(KT):
                    nc.tensor.transpose(
                        pst[:, kt, :],
                        hh[:, di, kt, b, base + wc * 128: base + (wc + 1) * 128],
                        ident)
                stage = stg.tile([128, KT * 128], f32)
                if (b + wc) % 2 == 0:
                    nc.scalar.copy(stage, pst)
                else:
                    nc.vector.tensor_copy(stage, pst)
                nc.sync.dma_start(
                    out=odram.ap()[b, wc * 128:(wc + 1) * 128, :],
                    in_=stage)

            for b in range(BC):
                emit_out(0, b, 0)
                emit_out(1, b, 1)

            for t in range(W // 2, W):
                st_f = scan_mm(t, 0)
                if pend_b is not None:
                    scan_ew(pend_b)
                scan_ew(st_f)
                pend_b = scan_mm(t, 1)
            scan_ew(pend_b)

            # ---- Phase D (part 2): remaining output chunks ----
            for b in range(BC):
                emit_out(0, b, 1)
                emit_out(1, b, 0)

    nc.compile()
    return nc


def get_nc():
    if "nc" not in _NC_CACHE:
        _NC_CACHE["nc"] = build_nc()
    return _NC_CACHE["nc"]


def prep_inputs(hidden_states, w_ih_f, w_hh_f, b_f, w_ih_b, w_hh_b, b_b,
                word_ids):
    """Host-side layout/dtype prep. Returns per-core input maps."""
    bf16 = ml_dtypes.bfloat16
    hidden_states = np.ascontiguousarray(hidden_states, dtype=np.float32)
    word_ids = np.asarray(word_ids)

    # scaled one-hot from the (index-only) word_ids
    M = (word_ids[:, :, None] == np.arange(W, dtype=word_ids.dtype)[None, None, :])
    M = M.astype(np.float32)
    counts = M.sum(axis=1)
    M *= (1.0 / np.maximum(counts, 1.0))[:, None, :]

    # gate permutation [i, f, g, o] -> [i, f, o, g]
    perm = np.concatenate([np.arange(0, 512), np.arange(768, 1024),
                           np.arange(512, 768)])

    def prep_dir(w_ih, w_hh, b):
        w_ih = np.asarray(w_ih, dtype=np.float32)[:, perm]
        w_hh = np.asarray(w_hh, dtype=np.float32)[:, perm]
        b = np.asarray(b, dtype=np.float32)[perm]
        return (w_ih.reshape(ND, 128, G),
                w_hh.reshape(KT, 128, G).astype(bf16),
                b.reshape(NG, 128))

    wf, whf, bf_ = prep_dir(w_ih_f, w_hh_f, b_f)
    wb, whb, bb_ = prep_dir(w_ih_b, w_hh_b, b_b)
    wih_all = np.ascontiguousarray(np.stack([wf, wb]))
    whh_all = np.ascontiguousarray(np.stack([whf, whb]))
    bias_all = np.ascontiguousarray(np.concatenate([bf_, bb_], axis=0))

    in_maps = []
    for c in range(NCORES):
        sl = slice(c * BC, (c + 1) * BC)
        in_maps.append({
            "hs": np.ascontiguousarray(
                hidden_states[sl].reshape(BC, NT, 128, D)),
            "msc": np.ascontiguousarray(M[sl].reshape(BC, NT, 128, W)),
            "wih": wih_all,
            "whh": whh_all,
            "bias": bias_all,
        })
    return in_maps


def assemble_output(results):
    out = np.empty((NCORES * BC, W, 2 * H), dtype=np.float32)
    for c, r in enumerate(results):
        sl = slice(c * BC, (c + 1) * BC)
        out[sl, :, :H] = r["outf"]
        out[sl, :, H:] = r["outb"]
    return out


def kernel(hidden_states, w_ih_f, w_hh_f, b_f, w_ih_b, w_hh_b, b_b,
           word_ids, max_seq_len=None, **_unused):
    from concourse.bass_utils import run_bass_kernel_spmd

    in_maps = prep_inputs(hidden_states, w_ih_f, w_hh_f, b_f,
                          w_ih_b, w_hh_b, b_b, word_ids)
    nc = get_nc()
    res = run_bass_kernel_spmd(nc, in_maps, list(range(NCORES)))
    _NC_CACHE["last_exec_time_ns"] = res.exec_time_ns
    return assemble_output(res.results)


# revision 12
# speedup vs baseline: 1.0961x; 1.0961x over previous
"""Trainium2 Bass kernel for nn_Bert_BiLSTM (segment-mean pooling + BiLSTM).

Strategy (8 NeuronCores, data-parallel over batch, Bc=8 per core):
  Phase A (pooling): pooledT[d,w] = hidden[t,d]^T @ M_scaled[t,w] via fp32r
      matmuls, where M_scaled is the host-built one-hot(word_ids)/count
      matrix (index preprocessing only).
  Phase B (projection): pre[g,w] = w_ih^T @ pooledT (fp32r) + bias, stored
      bf16. Computed in w-halves ordered so the scan can start after the
      first two halves; the last two halves overlap the scan.
  Phase C (scan): 256 sequential LSTM steps per direction, both directions
      interleaved on each core. Gates in [G-part, B-free] layout: w_hh
      tiles (bf16) stationary, h (bf16) moving. pre_t is injected into
      PSUM via an identity matmul (opens the accumulation group), so the
      gates emerge complete in PSUM and ACT reads them directly.
      Elementwise in fp32 on DVE/ACT/GpSimd.
  Phase D: PE-transpose h history to [w, h] layout and DMA out.

Host side: shard batch, build M_scaled, permute gates to [i,f,o,g] order,
cast weights, assemble/concat outputs.
"""

import os
import sys

for _p in ("/opt/trn_rl_repo", "/root/.axon_site/_ro/trn_rl_repo"):
    if os.path.isdir(_p) and _p not in sys.path:
        sys.path.append(_p)

import numpy as np
import ml_dtypes

NCORES = 8
BC = 8          # batch per core
T = 512
D = 768
W = 256
H = 256
G = 1024        # 4*H
NT = T // 128   # 4 t-tiles
ND = D // 128   # 6 d-chunks
NG = G // 128   # 8 gate chunks (per direction)
KT = H // 128   # 2 h-chunks

_NC_CACHE = {}


def build_nc():
    """Build and compile the per-core Bass program (SPMD, same on all cores)."""
    import concourse.bacc as bacc
    import concourse.tile as tile
    from concourse import mybir
    from concourse.masks import make_identity

    f32 = mybir.dt.float32
    f32r = mybir.dt.float32r
    bf16 = mybir.dt.bfloat16
    AF = mybir.ActivationFunctionType
    ALU = mybir.AluOpType

    nc = bacc.Bacc("TRN2", target_bir_lowering=False, debug=False,
                   enable_asserts=False, num_devices=NCORES)

    hs = nc.dram_tensor("hs", [BC, NT, 128, D], f32r, kind="ExternalInput")
    msc = nc.dram_tensor("msc", [BC, NT, 128, W], f32r, kind="ExternalInput")
    wih = nc.dram_tensor("wih", [2, ND, 128, G], f32r, kind="ExternalInput")
    whh = nc.dram_tensor("whh", [2, KT, 128, G], bf16, kind="ExternalInput")
    bias = nc.dram_tensor("bias", [2 * NG, 128], f32, kind="ExternalInput")
    outf = nc.dram_tensor("outf", [BC, W, H], f32, kind="ExternalOutput")
    outb = nc.dram_tensor("outb", [BC, W, H], f32, kind="ExternalOutput")

    with tile.TileContext(nc) as tc:
        from contextlib import ExitStack
        ctx = ExitStack()
        with ctx:
            const = ctx.enter_context(tc.tile_pool(name="const", bufs=1))
            whh_sb = const.tile([128, 2, KT, G], bf16)
            nc.sync.dma_start(out=whh_sb, in_=whh.ap().rearrange("d k p g -> p d k g"))
            bias_sb = const.tile([128, 2 * NG], f32)
            nc.sync.dma_start(out=bias_sb, in_=bias.ap().rearrange("n p -> p n"))
            ident = const.tile([128, 128], bf16)
            make_identity(nc, ident)
            ident_pre = const.tile([128, 128], bf16)
            make_identity(nc, ident_pre)

            pooledT = const.tile([128, BC, ND, W], f32r)    # 48KB/part
            pre = const.tile([128, 2, W, NG, BC], bf16)     # 64KB/part
            hh = const.tile([128, 2, KT, BC, W + 1], bf16)  # h history
            cc = const.tile([128, 2, KT, BC], f32)

            # ---- Phase A: pooling ----
            with tc.tile_pool(name="hsst", bufs=2) as hsp, \
                 tc.tile_pool(name="mscst", bufs=2) as mscp, \
                 tc.tile_pool(name="psA", bufs=6, space="PSUM") as psA:
                for b in range(BC):
                    hst = []
                    msct = []
                    for tt in range(NT):
                        ht = hsp.tile([128, D], f32r, tag=f"hs{tt}")
                        nc.sync.dma_start(out=ht, in_=hs.ap()[b, tt])
                        hst.append(ht)
                        mt = mscp.tile([128, W], f32r, tag=f"ms{tt}")
                        nc.sync.dma_start(out=mt, in_=msc.ap()[b, tt])
                        msct.append(mt)
                    for dc in range(ND):
                        pps = psA.tile([128, W], f32)
                        for tt in range(NT):
                            nc.tensor.matmul(
                                out=pps,
                                lhsT=hst[tt][:, dc * 128:(dc + 1) * 128],
                                rhs=msct[tt],
                                start=(tt == 0), stop=(tt == NT - 1))
                        if (b * ND + dc) % 2 == 0:
                            nc.scalar.copy(pooledT[:, b, dc, :], pps)
                        else:
                            nc.vector.tensor_copy(pooledT[:, b, dc, :], pps)

            # scan pools first so the proj/psD pool stacks can close in
            # LIFO order around them
            bc_ctx = ctx.enter_context(ExitStack())
            psC = bc_ctx.enter_context(tc.tile_pool(name="psC", bufs=3, space="PSUM"))
            sp = bc_ctx.enter_context(tc.tile_pool(name="sp", bufs=3))
            gp = bc_ctx.enter_context(tc.tile_pool(name="gp", bufs=3))
            tp = bc_ctx.enter_context(tc.tile_pool(name="tp", bufs=3))
            thp = bc_ctx.enter_context(tc.tile_pool(name="thp", bufs=3))

            # ---- Phase B: projection in w-halves; scan interleaved ----
            pb_ctx = ExitStack()
            wihp = pb_ctx.enter_context(tc.tile_pool(name="wihp", bufs=1))
            psB = pb_ctx.enter_context(tc.tile_pool(name="psB", bufs=2, space="PSUM"))

            def proj_half(di, wh):
                wih_sb = wihp.tile([128, ND, G], f32r, tag="wih")
                nc.sync.dma_start(
                    out=wih_sb, in_=wih.ap()[di].rearrange("c p g -> p c g"))
                for gc in range(NG):
                    for bq in range(2):
                        ppj = psB.tile([128, 4, 128], f32)   # 1 bank
                        for dc in range(ND):
                            nc.tensor.matmul(
                                out=ppj,
                                lhsT=wih_sb[:, dc, gc * 128:(gc + 1) * 128],
                                rhs=pooledT[:, 4 * bq:4 * bq + 4, dc,
                                            wh * 128:(wh + 1) * 128],
                                start=(dc == 0), stop=(dc == ND - 1))
                        bcol = bias_sb[:, di * NG + gc: di * NG + gc + 1]
                        # pre is (w, gc, b)-ordered; psum is (b, w)
                        dst = pre[:, di, wh * 128:(wh + 1) * 128, gc,
                                  4 * bq:4 * bq + 4]
                        src_ap = ppj.rearrange("p b w -> p w b")
                        if (gc + bq) % 2 == 0:
                            nc.scalar.activation(dst, src_ap, AF.Identity,
                                                 bias=bcol, scale=1.0)
                        else:
                            nc.vector.tensor_scalar(dst, src_ap, bcol, None,
                                                    ALU.add)

            # ---- Phase C: the LSTM scan ----
            nc.vector.memset(hh[:, 0, :, :, 0], 0.0)     # fwd h_{-1} = 0
            nc.vector.memset(hh[:, 1, :, :, W], 0.0)     # bwd h_{W} = 0
            nc.vector.memset(cc, 0.0)

            def scan_mm(t, di):
                tf = t if di == 0 else W - 1 - t
                rslot = tf if di == 0 else tf + 1
                wslot = tf + 1 if di == 0 else tf
                # g gates (tanh) in their own bank so tanh starts early
                ps_g = psC.tile([128, KT, BC], f32, tag="psg")
                nc.tensor.matmul(out=ps_g, lhsT=ident_pre,
                                 rhs=pre[:, di, tf, 6:8, :],
                                 start=True, stop=False)
                # kt-outer: all k0 matmuls first so they're gated only on
                # the h0 half-write of the previous step
                for kt in range(KT):
                    for j, gc in enumerate((6, 7)):
                        nc.tensor.matmul(
                            out=ps_g[:, j, :],
                            lhsT=whh_sb[:, di, kt, gc * 128:(gc + 1) * 128],
                            rhs=hh[:, di, kt, :, rslot],
                            start=False, stop=(gc == 7 and kt == KT - 1))
                # i,f,o gates (sigmoid) in a second bank
                ps_s = psC.tile([128, 6, BC], f32, tag="psifo")
                nc.tensor.matmul(out=ps_s, lhsT=ident_pre,
                                 rhs=pre[:, di, tf, 0:6, :],
                                 start=True, stop=False)
                for kt in range(KT):
                    for gc in range(6):
                        nc.tensor.matmul(
                            out=ps_s[:, gc, :],
                            lhsT=whh_sb[:, di, kt, gc * 128:(gc + 1) * 128],
                            rhs=hh[:, di, kt, :, rslot],
                            start=False, stop=(gc == 5 and kt == KT - 1))
                return (di, ps_g, ps_s, wslot)

            def scan_ew(st):
                di, ps_g, ps_s, wslot = st
                g = gp.tile([128, KT, BC], f32)
                nc.scalar.activation(g, ps_g, AF.Tanh)
                s = sp.tile([128, 6, BC], f32)
                nc.scalar.activation(s, ps_s, AF.Sigmoid)
                tmp = tp.tile([128, KT, BC], f32)
                nc.gpsimd.tensor_mul(tmp, s[:, 0:2, :], g)
                nc.vector.tensor_mul(cc[:, di], s[:, 2:4, :], cc[:, di])
                nc.vector.tensor_add(cc[:, di], cc[:, di], tmp)
                th = thp.tile([128, KT, BC], f32)
                nc.scalar.activation(th, cc[:, di], AF.Tanh)
                # split h write by kt half: next step's k0 matmuls only wait
                # for the first half
                nc.vector.tensor_mul(hh[:, di, 0, :, wslot], s[:, 4, :], th[:, 0, :])
                nc.vector.tensor_mul(hh[:, di, 1, :, wslot], s[:, 5, :], th[:, 1, :])

            # Anti-phase emission: bwd's elementwise is emitted alongside
            # fwd's matmul burst and vice versa, so each chain's PE burst
            # hides under the other chain's ACT/DVE tail.
            pend_b = None
            proj_half(0, 0)   # fwd, w in [0,128)
            proj_half(1, 1)   # bwd, w in [128,256)
            for t in range(W // 2):
                st_f = scan_mm(t, 0)
                if pend_b is not None:
                    scan_ew(pend_b)
                scan_ew(st_f)
                pend_b = scan_mm(t, 1)
            scan_ew(pend_b)
            pend_b = None
            proj_half(0, 1)
            proj_half(1, 0)
            pb_ctx.close()

            # ---- Phase D (part 1): output chunks finished by scan half 1,
            # emitted here so they fill engine gaps during scan half 2 ----
            psD = bc_ctx.enter_context(tc.tile_pool(name="psD", bufs=2, space="PSUM"))
            stg = bc_ctx.enter_context(tc.tile_pool(name="stg", bufs=4))

            def emit_out(di, b, wc):
                odram = outf if di == 0 else outb
                base = 1 if di == 0 else 0
                pst = psD.tile([128, KT, 128], bf16)
                for kt in range(KT):
                    nc.tensor.transpose(
                        pst[:, kt, :],
                        hh[:, di, kt, b, base + wc * 128: base + (wc + 1) * 128],
                        ident)
                stage = stg.tile([128, KT * 128], f32)
                if (b + wc) % 2 == 0:
                    nc.scalar.copy(stage, pst)
                else:
                    nc.vector.tensor_copy(stage, pst)
                nc.sync.dma_start(
                    out=odram.ap()[b, wc * 128:(wc + 1) * 128, :],
                    in_=stage)

            for b in range(BC):
                emit_out(0, b, 0)
                emit_out(1, b, 1)

            for t in range(W // 2, W):
                st_f = scan_mm(t, 0)
                if pend_b is not None:
                    scan_ew(pend_b)
                scan_ew(st_f)
                pend_b = scan_mm(t, 1)
            scan_ew(pend_b)

            # ---- Phase D (part 2): remaining output chunks ----
            for b in range(BC):
                emit_out(0, b, 1)
                emit_out(1, b, 0)

    nc.compile()
    return nc


def get_nc():
    if "nc" not in _NC_CACHE:
        _NC_CACHE["nc"] = build_nc()
    return _NC_CACHE["nc"]


def prep_inputs(hidden_states, w_ih_f, w_hh_f, b_f, w_ih_b, w_hh_b, b_b,
                word_ids):
    """Host-side layout/dtype prep. Returns per-core input maps."""
    bf16 = ml_dtypes.bfloat16
    hidden_states = np.ascontiguousarray(hidden_states, dtype=np.float32)
    word_ids = np.asarray(word_ids)

    # scaled one-hot from the (index-only) word_ids
    M = (word_ids[:, :, None] == np.arange(W, dtype=word_ids.dtype)[None, None, :])
    M = M.astype(np.float32)
    counts = M.sum(axis=1)
    M *= (1.0 / np.maximum(counts, 1.0))[:, None, :]

    # gate permutation [i, f, g, o] -> [i, f, o, g]
    perm = np.concatenate([np.arange(0, 512), np.arange(768, 1024),
                           np.arange(512, 768)])

    def prep_dir(w_ih, w_hh, b):
        w_ih = np.asarray(w_ih, dtype=np.float32)[:, perm]
        w_hh = np.asarray(w_hh, dtype=np.float32)[:, perm]
        b = np.asarray(b, dtype=np.float32)[perm]
        return (w_ih.reshape(ND, 128, G),
                w_hh.reshape(KT, 128, G).astype(bf16),
                b.reshape(NG, 128))

    wf, whf, bf_ = prep_dir(w_ih_f, w_hh_f, b_f)
    wb, whb, bb_ = prep_dir(w_ih_b, w_hh_b, b_b)
    wih_all = np.ascontiguousarray(np.stack([wf, wb]))
    whh_all = np.ascontiguousarray(np.stack([whf, whb]))
    bias_all = np.ascontiguousarray(np.concatenate([bf_, bb_], axis=0))

    in_maps = []
    for c in range(NCORES):
        sl = slice(c * BC, (c + 1) * BC)
        in_maps.append({
            "hs": np.ascontiguousarray(
                hidden_states[sl].reshape(BC, NT, 128, D)),
            "msc": np.ascontiguousarray(M[sl].reshape(BC, NT, 128, W)),
            "wih": wih_all,
            "whh": whh_all,
            "bias": bias_all,
        })
    return in_maps


def assemble_output(results):
    out = np.empty((NCORES * BC, W, 2 * H), dtype=np.float32)
    for c, r in enumerate(results):
        sl = slice(c * BC, (c + 1) * BC)
        out[sl, :, :H] = r["outf"]
        out[sl, :, H:] = r["outb"]
    return out


def kernel(hidden_states, w_ih_f, w_hh_f, b_f, w_ih_b, w_hh_b, b_b,
           word_ids, max_seq_len=None, **_unused):
    from concourse.bass_utils import run_bass_kernel_spmd

    in_maps = prep_inputs(hidden_states, w_ih_f, w_hh_f, b_f,
                          w_ih_b, w_hh_b, b_b, word_ids)
    nc = get_nc()
    res = run_bass_kernel_spmd(nc, in_maps, list(range(NCORES)))
    _NC_CACHE["last_exec_time_ns"] = res.exec_time_ns
    return assemble_output(res.results)
